# revision 20
# baseline (speedup 1.0000x reference)
"""Styled 3D conv (StyleGAN-style modulated conv3d) on 8 Trainium2 NeuronCores.

Reference computation:
  st = s @ style_weight.T + style_bias                 (N, Cin)
  w  = weight[None] * st[:, None, :, None*3]           (N, Cout, Cin, 3,3,3)
  w  = w * rsqrt(sum(w^2 over (Cin,kd,kh,kw)) + eps)   per-sample demodulated
  y  = grouped_conv3d(x, w, groups=N, VALID) + bias    (N, Cout, 62,62,62)

Shapes: x (4, 64, 64, 64, 64) f32, y (4, 64, 62, 62, 62) f32.

Strategy (8 cores = 4 samples x 2 depth-halves):
  - Host: compute the tiny per-sample modulated weights (fp32), pack into
    matmul lhsT blocks, cast x to bf16.
  - Each core gets 34 input depth-planes and produces 31.5 output planes:
    the odd core of each sample receives its shard depth+height MIRRORED
    (with kd/kh taps mirrored in its lhsT), so one uniform SPMD program
    splits the 62 output planes 31/31; the half-covered boundary pair is
    merged on gather.
  - SBUF x layout: "slot" q = [128 partitions, 64, 64] with partitions
    0:64 = (ci, even plane 2q), 64:128 = (ci, odd plane 2q+1).
  - Output plane pair (2j, 2j+1) is computed with M=128 packing (psum
    partitions 0:64 -> y[2j] Cout, 64:128 -> y[2j+1]) against K=128 =
    (2 planes x ci) lhsT blocks: slot j ("var0"/A) + slot j+1 ("var1"/B)
    per (kh, kw) tap; 6/8 active 64x64 quadrants = 75% PE utilization
    (optimal for this shape at K=128).
  - fp8 hybrid: 2 of the 9 (kh,kw) taps run as fp8e4m3 DoubleRow matmuls
    (K=256: both slots in one instruction via a [128, 2, rows, 62] rhs AP
    over a contiguous fp8 slot array; one DR MM costs ~1 bf16 MM = 2x
    throughput). x is pre-scaled by 1/4 and the fp8 lhsT by 4, so fp8
    products accumulate into the same PSUM at the true scale. 16
    matmuls/chunk instead of 18 at rel-err ~1.65e-2 (gate 2e-2).
  - The fp8 x copy is derived ON-CHIP: the (idle) scalar engine converts
    each bf16 slot with activation(Copy, scale=1/4) into a contiguous
    fp8 slot array. This halves the input HBM traffic vs shipping a
    second x copy; under full-8-core load the extra DMA was tipping the
    chip into a 2.0 GHz throttle (vs 2.4 GHz nominal) that slowed every
    PE instruction by exactly 1.2x.
  - y is stored as bf16 (host converts back to f32): halves output HBM
    traffic for the same throttle reason; adds ~0.1% rms rounding, far
    inside the error budget.
  - Per pair, the 62x62 spatial output is split into 8 PSUM banks
    (7 chunks of 8 rows + 1 of 6); CHUNK-major issue order: all 16
    streams of a chunk back-to-back, then that bank drains on DVE while
    the next chunk accumulates. (Weight reloads are free: FWL + the PE's
    64-deep reorder window hide per-MM LDWEIGHTS at N=496 regardless of
    order; chunk-major kills the pair-boundary drain-chase stalls that
    stream-major had.) Streams are ordered bf16-first / fp8-last so the
    first pair can start before the fp8 conversions finish.
  - Drain: DVE tensor_scalar_add(psum, bias) -> bf16 SBUF staging -> DMA.
  - bf16 x slots stream through a 6-buffer sliding window.
"""

import numpy as np
import ml_dtypes

import concourse.mybir as mybir
import concourse.tile as tile
from concourse import bacc
from concourse.bass_utils import run_bass_kernel_spmd

EPS = 1e-8
N_CORES = 8
N, CIN, COUT, D = 4, 64, 64, 64
DO = D - 2              # 62 output planes/rows/cols
PLANES_IN = 34          # input planes per core
PAIRS = 16              # output plane pairs per core (32 planes)
SLOTS = PLANES_IN // 2  # 17
ROWS_PER_CHUNK = 8
CHUNKS = 8              # 7*8 + 6 = 62 rows
BF16 = mybir.dt.bfloat16
FP8 = mybir.dt.float8e4
F32 = mybir.dt.float32
DR = mybir.MatmulPerfMode.DoubleRow

FP8_TAPS = [(0, 0), (2, 2)]            # (kh, kw) taps computed in fp8
BF16_TAPS = [(kh, kw) for kh in range(3) for kw in range(3)
             if (kh, kw) not in FP8_TAPS]
FP8_SCALE = 4.0                        # x/4 in fp8, lhsT*4 -> exact products
N_BF16 = 2 * len(BF16_TAPS)            # 14 bf16 streams (issued first)
N_STREAMS = N_BF16 + len(FP8_TAPS)     # 16
XBUFS = 6                              # bf16 slot sliding window (pool bufs)
PRELOAD = 2                            # slots loaded before the pair loop:
                                       # a smaller head burst keeps the
                                       # chip-level DMA spike (x8 cores)
                                       # below the compute-clock throttle
WARM_MMS = 100                         # PE warmup matmuls (clock ramp + head)

_compiled = {}


def _raw_matmul(te, out, lhsT, rhs, start, stop, perf_mode):
    """BassTensorEngine.matmul body for DoubleRow with a 4D rhs AP (the
    public wrapper only differs in shape bookkeeping; dtypes here are the
    supported fp8e4)."""
    keep_dims = {0, 1}
    ifmap_ap = te.lower_ap(rhs.opt(keep_dims), opt=False)
    weights_ap = te.lower_ap(lhsT.opt(keep_dims), opt=False,
                             for_matmul_weights=True)
    out_ap = te.lower_ap(out)
    tile_size = (128, 128)
    tile_position = (lhsT.base_partition(), out.base_partition())
    return te.add_instruction(mybir.InstMatmult(
        name=te.bass.get_next_instruction_name(),
        replication_resolution=0, replication_shift_amnt=0,
        replication_num_rows=0,
        start_tensor_calc=start, stop_tensor_calc=stop,
        ins=[ifmap_ap, weights_ap], outs=[out_ap],
        perf_mode=perf_mode, is_transpose=None,
        ifmap_quant_offset=None, weights_quant_offset=None,
        bass_skip_group_check=False,
        tile_position=tile_position, tile_size=tile_size))


def _build_nc():
    nc = bacc.Bacc("TRN2", target_bir_lowering=False, debug=False,
                   num_devices=N_CORES)
    # xs is plane-major on the DRAM side: row (q*64 + ci) = plane q, chan
    # ci. Slot loads then read CONTIGUOUS 1 MB ranges; the previous
    # chan-major layout made each slot load 64 scattered 8 KB reads, and
    # that strided HBM pattern (x8 cores) tripped a chip-level power
    # throttle that clocked PE+DVE down 1.2x for the whole run.
    xs = nc.dram_tensor("xs", [PLANES_IN * CIN, D * D], BF16,
                        kind="ExternalInput").ap()
    wtsb = nc.dram_tensor("wtsb", [128, 14 * 128], BF16,
                          kind="ExternalInput").ap()
    wts8 = nc.dram_tensor("wts8", [128, 4 * 128], FP8,
                          kind="ExternalInput").ap()
    b128 = nc.dram_tensor("b128", [128, 1], F32, kind="ExternalInput").ap()
    # y is plane-major too: row (plane*64 + cout). A pair store is then a
    # single contiguous [128, 3844] DMA (partitions 0:64 = plane 2j,
    # 64:128 = plane 2j+1), sequential on the DRAM side.
    y = nc.dram_tensor("y", [2 * PAIRS * COUT, DO * DO], BF16,
                       kind="ExternalOutput").ap()

    with tile.TileContext(nc) as tc:
        with (
            tc.tile_pool(name="wp", bufs=1) as wpool,
            tc.tile_pool(name="xp", bufs=XBUFS) as xpool,
            tc.tile_pool(name="x8p", bufs=1) as x8pool,
            tc.tile_pool(name="ps", bufs=CHUNKS, space="PSUM") as pspool,
            tc.tile_pool(name="st", bufs=2) as stpool,
        ):
            # PE warmup source: memset on GpSimd (the least-loaded engine)
            # so the warmup matmuls can start right after the boot barrier.
            warm_src = wpool.tile([128, 496], BF16, name="warm_src")
            nc.gpsimd.memset(warm_src[:, :], 0.0)
            wb_sb = wpool.tile([128, 14, 128], BF16)
            w8_sb = wpool.tile([128, 2, 2, 128], FP8)
            bias_sb = wpool.tile([128, 1], F32)

            # fp8 x: one contiguous slot array so a [128, 2, rows, 62]
            # DoubleRow rhs AP can step across adjacent slots. Filled
            # on-chip from the bf16 slots by the scalar engine.
            xf8 = x8pool.tile([128, SLOTS, D, D], FP8)

            # bf16 x slots: sliding window of XBUFS tiles
            slots = [None] * SLOTS

            def load_slot(q):
                t = xpool.tile([128, D, D], BF16, tag="xslot",
                               name=f"xslot_{q}")
                r0 = 2 * q * CIN
                nc.sync.dma_start(t[:, :, :].opt(), xs[r0:r0 + 128, :])
                slots[q] = t
                # derive the fp8 copy on the (otherwise idle) scalar engine
                nc.scalar.activation(xf8[:, q, :, :], t[:, :, :],
                                     mybir.ActivationFunctionType.Copy,
                                     scale=1.0 / FP8_SCALE)

            # head order: the data the first chunks consume goes first —
            # rows 0..15 of slots 0/1, then the weights, then the rest.
            t0 = xpool.tile([128, D, D], BF16, tag="xslot", name="xslot_0")
            t1 = xpool.tile([128, D, D], BF16, tag="xslot", name="xslot_1")
            slots[0], slots[1] = t0, t1
            for t, q in ((t0, 0), (t1, 1)):
                nc.sync.dma_start(t[:, 0:16, :].opt(),
                                  xs[2 * q * CIN:2 * q * CIN + 128, 0:16 * D])
            for i in range(4):
                o = i * 448
                nc.sync.dma_start(
                    wb_sb[:, :, :].opt()[:, o:o + 448],
                    wtsb[:, o:o + 448])
            nc.sync.dma_start(w8_sb[:, :, :, :].opt(), wts8[:, :])
            nc.sync.dma_start(bias_sb[:, :], b128[:, :])
            for t, q in ((t0, 0), (t1, 1)):
                for r in range(16, D, 16):
                    nc.sync.dma_start(t[:, r:r + 16, :].opt(),
                                      xs[2 * q * CIN:2 * q * CIN + 128,
                                         r * D:(r + 16) * D])
            for t, q in ((t0, 0), (t1, 1)):
                nc.scalar.activation(xf8[:, q, :, :], t[:, :, :],
                                     mybir.ActivationFunctionType.Copy,
                                     scale=1.0 / FP8_SCALE)
            for q in range(2, PRELOAD):
                load_slot(q)

            # PE warmup: the HAM clock gate holds the PE at 1.2 GHz until
            # ~3.4us of sustained activity; run junk matmuls on a scratch
            # bank while the first slots' DMAs are in flight.
            warm_ps = pspool.tile([128, 496], F32, tag="ps", name="warm_ps")
            for _ in range(WARM_MMS):
                nc.tensor.matmul(warm_ps[:, :64], warm_src[:, 0:128],
                                 warm_src[:, 128:192], start=True, stop=True)

            def issue_stream(s_idx, j, c, psum):
                """Matmul stream s_idx of pair j into psum bank for chunk c.
                Streams 0..13 are bf16 taps, 14..15 the fp8 DR taps."""
                rows = min(ROWS_PER_CHUNK, DO - c * ROWS_PER_CHUNK)
                r0 = c * ROWS_PER_CHUNK
                start = s_idx == 0
                stop = s_idx == N_STREAMS - 1
                # order: 7 bf16 (var0), DR tap0, 7 bf16 (var1), DR tap1 —
                # a DR weight load (256 cols, no FWL, no background buffer)
                # is only hidden when bf16 streams run before it; two
                # back-to-back DR MMs exposed ~397 ns per chunk.
                if s_idx in (7, 15):
                    t = s_idx // 8
                    kh, kw = FP8_TAPS[t]
                    _raw_matmul(
                        nc.tensor, psum[:, :rows * DO],
                        w8_sb[:, t, :, :],
                        xf8[:, j:j + 2, r0 + kh:r0 + kh + rows, kw:kw + DO],
                        start=start, stop=stop, perf_mode=DR)
                else:
                    b = s_idx - (s_idx > 7)
                    var, ti = divmod(b, len(BF16_TAPS))
                    kh, kw = BF16_TAPS[ti]
                    nc.tensor.matmul(
                        psum[:, :rows * DO], wb_sb[:, var * 7 + ti, :],
                        slots[j + var][:, r0 + kh:r0 + kh + rows, kw:kw + DO],
                        start=start, stop=stop)

            for j in range(PAIRS):
                half = j == PAIRS - 1
                # the half pair computes only rows 0..31 (chunks 0..3); the
                # mirrored partner core supplies the remaining rows.
                nchunks = 4 if half else CHUNKS
                if 1 <= j <= 13:
                    load_slot(j + 3)
                staging = stpool.tile([128, DO * DO], BF16, tag="stage")
                psums = [pspool.tile([128, 496], F32, tag="ps",
                                     name=f"ps_{j}_{c}")
                         for c in range(nchunks)]

                # chunk-major: bank c drains on DVE while chunk c+1
                # accumulates; no pair-boundary drain pile-up.
                for c in range(nchunks):
                    if j == 0 and c in (3, 6):
                        # pace the head DMA: slots 2/3 load mid-pair-0
                        # instead of in one up-front burst (power margin)
                        load_slot(2 if c == 3 else 3)
                    for s_idx in range(N_STREAMS):
                        issue_stream(s_idx, j, c, psums[c])
                    rows = min(ROWS_PER_CHUNK, DO - c * ROWS_PER_CHUNK)
                    n = rows * DO
                    o = c * ROWS_PER_CHUNK * DO
                    nc.vector.tensor_scalar_add(staging[:, o:o + n],
                                                psums[c][:, :n], bias_sb)
                    if half:
                        # early chunks ride the (idle) GpSimd SW queue; the
                        # tail-exposed final chunks take the HW queues
                        eng = (nc.gpsimd, nc.gpsimd, nc.sync, nc.scalar)[c]
                        eng.dma_start(
                            y[2 * j * COUT:(2 * j + 2) * COUT, o:o + n],
                            staging[:, o:o + n])
                if not half:
                    for p in range(0, 128, 32):
                        nc.scalar.dma_start(
                            y[2 * j * COUT + p:2 * j * COUT + p + 32, :],
                            staging[p:p + 32, :])
    nc.compile()
    return nc


def _modulated_weights(s_n, style_weight, style_bias, weight):
    st = s_n.astype(np.float32) @ style_weight.T.astype(np.float32) + style_bias
    w = weight * st[None, :, None, None, None]
    demod = 1.0 / np.sqrt(np.sum(w * w, axis=(1, 2, 3, 4)) + EPS)
    return w * demod[:, None, None, None, None]


def _build_lhsT(wmod):
    """(9, 2, 128, 128) fp32: [kh*3+kw, var]; lhsT[k=(half,ci), m=(colhalf,co)]."""
    out = np.zeros((9, 2, 128, 128), np.float32)
    for kh in range(3):
        for kw in range(3):
            b = kh * 3 + kw
            wt = wmod[:, :, :, kh, kw]         # (co, ci, kd)
            A = out[b, 0]
            B = out[b, 1]
            A[0:64, 0:64] = wt[:, :, 0].T      # lower -> y[d],   kd0
            A[64:128, 0:64] = wt[:, :, 1].T    # upper -> y[d],   kd1
            A[64:128, 64:128] = wt[:, :, 0].T  # upper -> y[d+1], kd0
            B[0:64, 0:64] = wt[:, :, 2].T      # lower -> y[d],   kd2
            B[0:64, 64:128] = wt[:, :, 1].T    # lower -> y[d+1], kd1
            B[64:128, 64:128] = wt[:, :, 2].T  # upper -> y[d+1], kd2
    return out


def _prepare_in_maps(x, s, style_weight, style_bias, weight, bias):
    bias128 = np.concatenate([bias.reshape(COUT), bias.reshape(COUT)])
    bias128 = np.ascontiguousarray(bias128.reshape(128, 1), np.float32)

    x_bf = x.astype(ml_dtypes.bfloat16)
    in_maps = []
    for core in range(N_CORES):
        n, half = divmod(core, 2)
        wmod = _modulated_weights(s[n], style_weight, style_bias, weight)
        if half == 0:
            xsl = x_bf[n][:, 0:PLANES_IN]
        else:
            # mirrored shard: flip depth + height; kernel taps flip too,
            # so the same program computes the flipped top half
            xsl = x_bf[n][:, D - PLANES_IN:D][:, ::-1, ::-1, :]
            wmod = wmod[:, :, ::-1, ::-1, :]
        # plane-major for contiguous slot DMAs: (plane, ci, h, w)
        xsl = xsl.transpose(1, 0, 2, 3).reshape(PLANES_IN * CIN, D * D)
        lhsT = _build_lhsT(np.ascontiguousarray(wmod))  # (9, 2, 128, 128)
        blks_bf = []
        for var in range(2):
            for kh, kw in BF16_TAPS:
                blks_bf.append(lhsT[kh * 3 + kw, var])
        wtsb = np.stack(blks_bf, axis=1).reshape(128, 14 * 128)
        wtsb = np.ascontiguousarray(wtsb).astype(ml_dtypes.bfloat16)
        blks_f8 = []
        for kh, kw in FP8_TAPS:
            for var in range(2):
                blks_f8.append(lhsT[kh * 3 + kw, var] * FP8_SCALE)
        wts8 = np.stack(blks_f8, axis=1).reshape(128, 4 * 128)
        wts8 = np.clip(np.ascontiguousarray(wts8), -240, 240)
        wts8 = wts8.astype(ml_dtypes.float8_e4m3)
        in_maps.append({"xs": np.ascontiguousarray(xsl),
                        "wtsb": wtsb, "wts8": wts8, "b128": bias128})
    return in_maps


def kernel(x, s, style_weight, style_bias, weight, bias):
    x = np.asarray(x)
    s = np.asarray(s)
    style_weight = np.asarray(style_weight, np.float32)
    style_bias = np.asarray(style_bias, np.float32)
    weight = np.asarray(weight, np.float32)
    bias = np.asarray(bias, np.float32)

    if "nc" not in _compiled:
        _compiled["nc"] = _build_nc()
    nc = _compiled["nc"]

    in_maps = _prepare_in_maps(x, s, style_weight, style_bias, weight, bias)
    res = run_bass_kernel_spmd(nc, in_maps, core_ids=list(range(N_CORES)))

    y = np.empty((N, COUT, DO, DO, DO), np.float32)
    for core in range(N_CORES):
        n, half = divmod(core, 2)
        ys = np.asarray(res.results[core]["y"]).astype(np.float32)
        ys = ys.reshape(2 * PAIRS, COUT, DO, DO).transpose(1, 0, 2, 3)
        if half == 0:
            # planes 0..29 full; planes 30,31 rows 0..31 only
            y[n][:, 0:30] = ys[:, 0:30]
            y[n][:, 30:32, 0:32] = ys[:, 30:32, 0:32]
        else:
            # un-mirror: ysf[p', r'] = global (plane 30+p', row r')
            ysf = ys[:, ::-1, ::-1, :]
            y[n][:, 32:DO] = ysf[:, 2:32]
            y[n][:, 30:32, 32:DO] = ysf[:, 0:2, 32:DO]
    return y
